# revision 23
# baseline (speedup 1.0000x reference)
"""Bass/Tile Trainium2 kernel for nn_AttentionSampling.

Problem: out = q + attention_downsampling(LN(q), LN(k), LN(v), factor=4)
  B=4, Sq=2048, Skv=8192, D=1024. Per query token s:
    w_f   = dot(LN(q)[s], LN(k)[4s+f])          f in 0..3  (no softmax)
    out[s] = q[s] + sum_f w_f * LN(v)[4s+f]

Key algebraic folding (valid for ln_weight==1, ln_bias==0, which is what
setup_inputs produces; a numpy fallback handles the general case):
    dot(LN(q), LN(k)) = aq*ak*(q.k - D*muq*muk)      a = rsqrt(var+eps)
    sum_f w_f*LN(v_f) = sum_f c_f*v_f - (sum_f c_f*muv_f)*ones,  c_f = w_f*av_f
so no normalized tensor is ever materialized: only raw dots + per-token stats.

v3: software-pipelined emission. HW probing showed the kernel is dependency-
chain bound, not engine-throughput bound: ACT/DVE/Pool have strict in-order
FIFOs, so tile t's late-chain instructions (writeback, rdot) emitted before
tile t+1's early work (bn_stats, q stats) block it -> near-serial per-tile
chains. Fix: emit stages skewed (loads t, DVE stats t-1, mid-chain t-2,
out-path t-3) so every FIFO sees only near-ready instructions.

Sharding: 8 cores = batch (4) x query-half (2). Each core owns 1024 windows:
q[1024,1024], k/v[1024,4,1024] (window-major view), out[1024,1024].
"""

import numpy as np


def _ensure_concourse():
    try:
        import concourse.bass  # noqa: F401
    except ImportError:
        import sys

        for p in ("/opt/trn_rl_repo", "/root/.axon_site/_ro/trn_rl_repo"):
            if p not in sys.path:
                sys.path.insert(0, p)


_ensure_concourse()

import concourse.bass as bass  # noqa: E402
import concourse.tile as tile  # noqa: E402
from concourse import mybir  # noqa: E402
from concourse.bass_utils import run_bass_kernel_spmd  # noqa: E402

# ---------------------------------------------------------------------------
# Walrus-compatibility shims.
#
# The walrus in this container rejects two things Tile's end-of-context tail
# emits: (a) the final Drain carrying >2 sem waits ("Too many sync wait
# commands"), and (b) EVENT_SEMAPHORE_RANGE_CLEAR ("ISA wrong length").
# Replace the tail with per-semaphore EventSemaphore instructions that wait
# for each sem's final value, then the normal all-engine barrier. A JSON-level
# pass additionally splits any instruction carrying more than MAX_WAITS sem
# waits into EventSemaphore wait carriers.
# ---------------------------------------------------------------------------

_MAX_WAITS = 1


def _patched_drain_and_barrier(self, tick_clock, wait_clock):
    nc = self.nc
    gc = tick_clock.global_clock
    sems = self.sems.allocated()  # proc idx -> SemaphoreHandle
    for proc in sorted(sems):
        h = sems[proc]
        if "DMA" not in h.name:
            continue  # engine sems are implied by stream completion
        final = int(gc[proc]) * 16
        if final > 0:
            nc.gpsimd.wait_ge(h, final)
    nc.all_engine_barrier()
    popped = nc._tile_sem_poison_stack.pop()
    assert popped is self._sem_poison


tile.TileContext._drain_and_barrier = _patched_drain_and_barrier

_orig_to_json_bytes = bass.Bass.to_json_bytes


def _to_json_bytes_compat(self):
    import orjson

    raw = _orig_to_json_bytes(self)
    d = orjson.loads(raw)
    changed = False
    for fn in d.get("functions", []):
        blocks = fn.get("basic_blocks") or fn.get("blocks") or []
        for bb in blocks:
            insts = bb.get("instructions", [])
            new_insts = []
            for inst in insts:
                waits = (inst.get("sync_info") or {}).get("on_wait") or []
                if len(waits) > _MAX_WAITS:
                    keep = waits[-_MAX_WAITS:]
                    excess = waits[:-_MAX_WAITS]
                    for i, wt in enumerate(excess):
                        new_insts.append(
                            {
                                "name": f"{inst['name']}_wsplit{i}",
                                "opcode": "EventSemaphore",
                                "engine": inst["engine"],
                                "ins": [],
                                "outs": [],
                                "debug": inst.get("debug"),
                                "sync_info": {"on_update": [], "on_wait": [wt]},
                            }
                        )
                    inst["sync_info"]["on_wait"] = keep
                    changed = True
                new_insts.append(inst)
            bb["instructions"] = new_insts
    return orjson.dumps(d) if changed else raw


bass.Bass.to_json_bytes = _to_json_bytes_compat

F32 = mybir.dt.float32
BF16 = mybir.dt.bfloat16
ALU = mybir.AluOpType
ACTF = mybir.ActivationFunctionType
AXL = mybir.AxisListType

B, SQ, SKV, D = 4, 2048, 8192, 1024
FACTOR = 4
N_CORES = 8
W_PER_CORE = B * SQ // N_CORES  # 1024 windows per core
P = 128  # windows per tile = SBUF partitions
LN_EPS = 1e-5
HALF = 512  # PSUM bank free-dim (f32)


def build_bass(n_tiles=W_PER_CORE // P, repeats=1, ablate=(), dma=None, skew=True):
    """repeats>1 wraps the body in a For_i hardware loop (timing NEFFs);
    repeats=1 is the straight-line correctness/production NEFF.
    ablate: timing-only probes that skip work (results become wrong):
      'ssq' | 'bn' | 'qstat' | 'outmm' | 'qkadd' | 'smalls'
    dma: queue map (q, k, v, out), entries 'sp' | 'act'.
    skew: software-pipelined stage emission (False = naive per-tile order).
    """
    if dma is None:
        dma = ("sp", "sp", "act", "act")
    dma_q, dma_k, dma_v, dma_o = dma
    nc = bass.Bass()
    q_d = nc.declare_dram_parameter("q", [n_tiles * P, D], BF16, isOutput=False)
    k_d = nc.declare_dram_parameter(
        "k", [n_tiles * P, FACTOR, D], BF16, isOutput=False
    )
    v_d = nc.declare_dram_parameter(
        "v", [n_tiles * P, FACTOR, D], BF16, isOutput=False
    )
    o_d = nc.declare_dram_parameter("out", [n_tiles * P, D], BF16, isOutput=True)
    id_d = nc.declare_dram_parameter("ident", [P, P], BF16, isOutput=False)

    lp = nc.allow_low_precision(reason="bf16 data/accums: rel_err gate is 2e-2")
    lp.__enter__()

    with tile.TileContext(nc) as tc:
        with (
            tc.tile_pool(name="qp", bufs=4) as qp,
            tc.tile_pool(name="kp", bufs=4) as kp,
            tc.tile_pool(name="vp", bufs=4) as vp,
            tc.tile_pool(name="qkp", bufs=3) as qkp,
            tc.tile_pool(name="outp", bufs=2) as outp,
            tc.tile_pool(name="scratch", bufs=2) as scratch,
            tc.tile_pool(name="smalls", bufs=4) as sm,
            tc.tile_pool(name="const", bufs=1) as cp,
            tc.tile_pool(name="psum", bufs=3, space="PSUM") as pp,
        ):
            ident = cp.tile([P, P], BF16)
            nc.sync.dma_start(ident[:], id_d[:])

            from contextlib import nullcontext

            def eng(which):
                return {"sp": nc.sync, "act": nc.scalar}[which]

            st = {}  # t -> per-tile state dict

            def stage_load(t):
                rows = slice(t * P, (t + 1) * P)
                s = st.setdefault(t, {})
                s["q"] = qp.tile([P, D], BF16, name="q", tag="q")
                eng(dma_q).dma_start(s["q"][:], q_d[rows, :])
                s["k"] = kp.tile([P, FACTOR, D], BF16, name="k", tag="k")
                eng(dma_k).dma_start(s["k"][:], k_d[rows, :, :])
                s["v"] = vp.tile([P, FACTOR, D], BF16, name="v", tag="v")
                eng(dma_v).dma_start(s["v"][:], v_d[rows, :, :])

            def stage_bn(t):
                """Front: DVE q+k adds then k/v bn_stats; ACT q-stats then
                S-squares (consuming the adds as they land)."""
                s = st[t]
                q_sb, k_sb, v_sb = s["q"], s["k"], s["v"]
                sum_q = sm.tile([P, 1], F32)
                ssq_q = sm.tile([P, 1], F32)
                if "qstat" not in ablate:
                    dmpq = scratch.tile([P, D], BF16, tag="actdump")
                    nc.scalar.activation(
                        dmpq[:], q_sb[:], ACTF.Copy, accum_out=sum_q[:]
                    )
                    dmpq2 = scratch.tile([P, D], BF16, tag="actdump")
                    nc.scalar.activation(
                        dmpq2[:], q_sb[:], ACTF.Square, accum_out=ssq_q[:]
                    )
                else:
                    nc.gpsimd.memset(sum_q[:], 0.0)
                    nc.gpsimd.memset(ssq_q[:], 1.0)
                s["sum_q"], s["ssq_q"] = sum_q, ssq_q

                qk = qkp.tile([P, FACTOR, D], BF16)
                if "qkadd" not in ablate:
                    for f in range(FACTOR):
                        nc.vector.tensor_tensor(
                            qk[:, f], k_sb[:, f], q_sb[:], ALU.add
                        )
                S = sm.tile([P, FACTOR], F32)
                if "ssq" not in ablate and "qkadd" not in ablate:
                    for f in range(FACTOR):
                        dmps = scratch.tile([P, D], BF16, tag="actdump")
                        nc.scalar.activation(
                            dmps[:], qk[:, f], ACTF.Square,
                            accum_out=S[:, f : f + 1],
                        )
                else:
                    nc.gpsimd.memset(S[:], 1.0)
                s["S"] = S

                bnst = sm.tile([P, 2, FACTOR, 2, 6], F32)
                stats = sm.tile([P, 2, FACTOR, 2], F32)  # (k/v, f, mean/var)
                if "bn" not in ablate:
                    for i, x_sb in ((0, k_sb), (1, v_sb)):
                        for f in range(FACTOR):
                            for ch in range(2):
                                nc.vector.bn_stats(
                                    bnst[:, i, f, ch],
                                    x_sb[:, f, ch * HALF : (ch + 1) * HALF],
                                )
                            nc.vector.bn_aggr(
                                stats[:, i, f],
                                bnst[:, i, f].rearrange("p c x -> p (c x)"),
                            )
                else:
                    nc.gpsimd.memset(stats[:], 0.5)
                s["stats"] = stats

            def stage_mid(t):
                """Small-tensor chain + diag builds (S/stats made last iter)."""
                s = st[t]
                stats = s["stats"]
                sum_q, ssq_q = s["sum_q"], s["ssq_q"]
                S = s["S"]
                mu_k = stats[:, 0, :, 0]
                mu_v = stats[:, 1, :, 0]

                mu_q = sm.tile([P, 1], F32)
                nc.gpsimd.tensor_scalar_mul(mu_q[:], sum_q[:], 1.0 / D)
                mmq = sm.tile([P, 1], F32)
                nc.gpsimd.tensor_mul(mmq[:], mu_q[:], mu_q[:])

                var_all = sm.tile([P, 9], F32)
                nc.gpsimd.tensor_scalar(
                    var_all[:, 0:1], ssq_q[:], 1.0 / D, mmq[:, 0:1],
                    ALU.mult, ALU.subtract,
                )
                nc.gpsimd.tensor_copy(var_all[:, 1:5], stats[:, 0, :, 1])
                nc.gpsimd.tensor_copy(var_all[:, 5:9], stats[:, 1, :, 1])
                veps = sm.tile([P, 9], F32)
                nc.gpsimd.tensor_scalar_add(veps[:], var_all[:], LN_EPS)
                rall = sm.tile([P, 9], F32)
                nc.vector.reciprocal(rall[:], veps[:])
                a_all = sm.tile([P, 9], F32)
                nc.scalar.sqrt(a_all[:], rall[:])
                aq = a_all[:, 0:1]
                ak = a_all[:, 1:5]
                av = a_all[:, 5:9]

                if "smalls" not in ablate:
                    mmk = sm.tile([P, FACTOR], F32)
                    nc.gpsimd.tensor_mul(mmk[:], mu_k, mu_k)
                    vpm = sm.tile([P, FACTOR], F32)
                    nc.gpsimd.tensor_tensor(
                        vpm[:], stats[:, 0, :, 1], mmk[:], ALU.add
                    )
                    # rdot = 0.5*(S - ssq_q) - 0.5*D*vpm
                    t_a = sm.tile([P, FACTOR], F32)
                    nc.gpsimd.tensor_scalar(
                        t_a[:], S[:], ssq_q[:, 0:1], 0.5, ALU.subtract, ALU.mult
                    )
                    rdot = sm.tile([P, FACTOR], F32)
                    nc.vector.scalar_tensor_tensor(
                        rdot[:], vpm[:], -0.5 * D, t_a[:], ALU.mult, ALU.add
                    )
                    # w_f = aq*ak_f*(rdot_f - D*muq*muk_f); c_f = w_f*av_f
                    t1 = sm.tile([P, FACTOR], F32)
                    nc.gpsimd.tensor_scalar(
                        t1[:], mu_k, mu_q[:, 0:1], None, ALU.mult
                    )
                    t2 = sm.tile([P, FACTOR], F32)
                    nc.vector.scalar_tensor_tensor(
                        t2[:], t1[:], -float(D), rdot[:], ALU.mult, ALU.add
                    )
                    u = sm.tile([P, FACTOR], F32)
                    nc.gpsimd.tensor_scalar(u[:], ak, aq, None, ALU.mult)
                    w = sm.tile([P, FACTOR], F32)
                    nc.gpsimd.tensor_mul(w[:], t2[:], u[:])
                    c = sm.tile([P, FACTOR], F32)
                    nc.gpsimd.tensor_mul(c[:], w[:], av)
                    e = sm.tile([P, FACTOR], F32)
                    nc.gpsimd.tensor_mul(e[:], c[:], mu_v)
                    neg_d = sm.tile([P, 1], F32)
                    nc.vector.tensor_reduce(
                        neg_d[:], e[:], AXL.X, ALU.add, negate=True
                    )
                else:
                    c = sm.tile([P, FACTOR], F32)
                    nc.gpsimd.tensor_scalar_mul(c[:], S[:], 1.0)
                    neg_d = sm.tile([P, 1], F32)
                    nc.gpsimd.memset(neg_d[:], 0.0)
                s["neg_d"] = neg_d

                diags = []
                for f in range(FACTOR):
                    dg = sm.tile([P, P], BF16, tag=f"diag{f}")
                    nc.gpsimd.tensor_scalar_mul(dg[:], ident[:], c[:, f : f + 1])
                    diags.append(dg)
                s["diags"] = diags

            def stage_out(t):
                """PE accumulation + ACT writeback + out DMA."""
                s = st.pop(t)
                q_sb, v_sb, diags = s["q"], s["v"], s["diags"]
                psum_t = pp.tile([P, 2, HALF], F32)
                if "outmm" not in ablate:
                    for h in range(2):
                        nc.tensor.matmul(
                            psum_t[:, h],
                            ident[:],
                            q_sb[:, h * HALF : (h + 1) * HALF],
                            start=True,
                            stop=False,
                        )
                    for f in range(FACTOR):
                        for h in range(2):
                            nc.tensor.matmul(
                                psum_t[:, h],
                                diags[f][:],
                                v_sb[:, f, h * HALF : (h + 1) * HALF],
                                start=False,
                                stop=(f == FACTOR - 1),
                            )
                else:
                    for h in range(2):
                        nc.tensor.matmul(
                            psum_t[:, h],
                            diags[0][:],
                            v_sb[:, 0, h * HALF : (h + 1) * HALF],
                            start=True,
                            stop=True,
                        )
                out_sb = outp.tile([P, D], BF16)
                nc.scalar.activation(
                    out_sb[:],
                    psum_t[:].rearrange("p c x -> p (c x)"),
                    ACTF.Identity,
                    bias=s["neg_d"][:],
                )
                rows = slice(t * P, (t + 1) * P)
                eng(dma_o).dma_start(o_d[rows, :], out_sb[:])

            loop_ctx = tc.For_i(0, repeats, 1) if repeats > 1 else nullcontext()
            with loop_ctx:
                if skew:
                    for i in range(n_tiles + 3):
                        if i < n_tiles:
                            stage_load(i)
                        if 0 <= i - 1 < n_tiles:
                            stage_bn(i - 1)
                        if 0 <= i - 2 < n_tiles:
                            stage_mid(i - 2)
                        if 0 <= i - 3 < n_tiles:
                            stage_out(i - 3)
                else:
                    for t in range(n_tiles):
                        stage_load(t)
                        stage_bn(t)
                        stage_mid(t)
                        stage_out(t)
    return nc


def make_in_map(q_core, k_core, v_core, layout="tile"):
    """Host-side per-core input prep shared by run()/test/sim: cast to bf16."""
    import ml_dtypes

    bf = ml_dtypes.bfloat16
    return {
        "q": np.ascontiguousarray(np.asarray(q_core, dtype=np.float32)).astype(bf),
        "k": np.ascontiguousarray(np.asarray(k_core, dtype=np.float32)).astype(bf),
        "v": np.ascontiguousarray(np.asarray(v_core, dtype=np.float32)).astype(bf),
        "ident": np.eye(P, dtype=np.float32).astype(bf),
    }


_NC_CACHE = None


def _get_nc():
    global _NC_CACHE
    if _NC_CACHE is None:
        _NC_CACHE = build_bass()
    return _NC_CACHE


def _numpy_reference(query, key, value, ln_w, ln_b):
    def ln(x):
        mu = x.mean(-1, keepdims=True)
        var = ((x - mu) ** 2).mean(-1, keepdims=True)
        return (x - mu) / np.sqrt(var + LN_EPS) * ln_w + ln_b

    qn, kn, vn = ln(query), ln(key), ln(value)
    b, s, d = key.shape
    k_win = kn.reshape(b, s // FACTOR, FACTOR, d)
    wts = np.einsum("bsd,bsfd->bsf", qn, k_win).reshape(b, s)
    attn = (wts[:, :, None] * vn).reshape(b, s // FACTOR, FACTOR, d).sum(axis=2)
    return (query + attn).astype(np.float32)


def run(inputs, trace=False):
    """Returns (full_output, BassKernelResults-or-None)."""
    query = np.asarray(inputs["query"], dtype=np.float32)
    key = np.asarray(inputs["key"], dtype=np.float32)
    value = np.asarray(inputs["value"], dtype=np.float32)
    ln_w = np.asarray(inputs["ln_weight"], dtype=np.float32)
    ln_b = np.asarray(inputs["ln_bias"], dtype=np.float32)

    if not (np.all(ln_w == 1.0) and np.all(ln_b == 0.0)):
        # General-path fallback (setup_inputs always produces ones/zeros).
        return _numpy_reference(query, key, value, ln_w, ln_b), None

    sq_h = SQ // 2  # 1024 query rows per core
    skv_h = SKV // 2  # 4096 kv rows per core
    in_maps = []
    for cidx in range(N_CORES):
        bi, h = divmod(cidx, 2)
        in_maps.append(
            make_in_map(
                query[bi, h * sq_h : (h + 1) * sq_h],
                key[bi, h * skv_h : (h + 1) * skv_h].reshape(W_PER_CORE, FACTOR, D),
                value[bi, h * skv_h : (h + 1) * skv_h].reshape(W_PER_CORE, FACTOR, D),
            )
        )

    res = run_bass_kernel_spmd(
        _get_nc(), in_maps, core_ids=list(range(N_CORES)), trace=trace
    )
    out = np.empty((B, SQ, D), dtype=np.float32)
    for cidx in range(N_CORES):
        bi, h = divmod(cidx, 2)
        out[bi, h * sq_h : (h + 1) * sq_h] = np.asarray(
            res.results[cidx]["out"], dtype=np.float32
        )
    return out, res


def kernel(**inputs) -> np.ndarray:
    out, _ = run(inputs)
    return out


# revision 25
# speedup vs baseline: 1.1149x; 1.1149x over previous
"""Bass/Tile Trainium2 kernel for nn_AttentionSampling.

Problem: out = q + attention_downsampling(LN(q), LN(k), LN(v), factor=4)
  B=4, Sq=2048, Skv=8192, D=1024. Per query token s:
    w_f   = dot(LN(q)[s], LN(k)[4s+f])          f in 0..3  (no softmax)
    out[s] = q[s] + sum_f w_f * LN(v)[4s+f]

Key algebraic folding (valid for ln_weight==1, ln_bias==0, which is what
setup_inputs produces; a numpy fallback handles the general case):
    dot(LN(q), LN(k)) = aq*ak*(q.k - D*muq*muk)      a = rsqrt(var+eps)
    sum_f w_f*LN(v_f) = sum_f c_f*v_f - (sum_f c_f*muv_f)*ones,  c_f = w_f*av_f
so no normalized tensor is ever materialized: only raw dots + per-token stats.

v3: software-pipelined emission. HW probing showed the kernel is dependency-
chain bound, not engine-throughput bound: ACT/DVE/Pool have strict in-order
FIFOs, so tile t's late-chain instructions (writeback, rdot) emitted before
tile t+1's early work (bn_stats, q stats) block it -> near-serial per-tile
chains. Fix: emit stages skewed (loads t, DVE stats t-1, mid-chain t-2,
out-path t-3) so every FIFO sees only near-ready instructions.

Sharding: 8 cores = batch (4) x query-half (2). Each core owns 1024 windows:
q[1024,1024], k/v[1024,4,1024] (window-major view), out[1024,1024].
"""

import numpy as np


def _ensure_concourse():
    try:
        import concourse.bass  # noqa: F401
    except ImportError:
        import sys

        for p in ("/opt/trn_rl_repo", "/root/.axon_site/_ro/trn_rl_repo"):
            if p not in sys.path:
                sys.path.insert(0, p)


_ensure_concourse()

import concourse.bass as bass  # noqa: E402
import concourse.tile as tile  # noqa: E402
from concourse import mybir  # noqa: E402
from concourse.bass_utils import run_bass_kernel_spmd  # noqa: E402

# ---------------------------------------------------------------------------
# Walrus-compatibility shims.
#
# The walrus in this container rejects two things Tile's end-of-context tail
# emits: (a) the final Drain carrying >2 sem waits ("Too many sync wait
# commands"), and (b) EVENT_SEMAPHORE_RANGE_CLEAR ("ISA wrong length").
# Replace the tail with per-semaphore EventSemaphore instructions that wait
# for each sem's final value, then the normal all-engine barrier. A JSON-level
# pass additionally splits any instruction carrying more than MAX_WAITS sem
# waits into EventSemaphore wait carriers.
# ---------------------------------------------------------------------------

_MAX_WAITS = 1


def _patched_drain_and_barrier(self, tick_clock, wait_clock):
    nc = self.nc
    gc = tick_clock.global_clock
    sems = self.sems.allocated()  # proc idx -> SemaphoreHandle
    for proc in sorted(sems):
        h = sems[proc]
        if "DMA" not in h.name:
            continue  # engine sems are implied by stream completion
        final = int(gc[proc]) * 16
        if final > 0:
            nc.gpsimd.wait_ge(h, final)
    nc.all_engine_barrier()
    popped = nc._tile_sem_poison_stack.pop()
    assert popped is self._sem_poison


tile.TileContext._drain_and_barrier = _patched_drain_and_barrier

_orig_to_json_bytes = bass.Bass.to_json_bytes


def _to_json_bytes_compat(self):
    import orjson

    raw = _orig_to_json_bytes(self)
    d = orjson.loads(raw)
    changed = False
    for fn in d.get("functions", []):
        blocks = fn.get("basic_blocks") or fn.get("blocks") or []
        for bb in blocks:
            insts = bb.get("instructions", [])
            new_insts = []
            for inst in insts:
                waits = (inst.get("sync_info") or {}).get("on_wait") or []
                if len(waits) > _MAX_WAITS:
                    keep = waits[-_MAX_WAITS:]
                    excess = waits[:-_MAX_WAITS]
                    for i, wt in enumerate(excess):
                        new_insts.append(
                            {
                                "name": f"{inst['name']}_wsplit{i}",
                                "opcode": "EventSemaphore",
                                "engine": inst["engine"],
                                "ins": [],
                                "outs": [],
                                "debug": inst.get("debug"),
                                "sync_info": {"on_update": [], "on_wait": [wt]},
                            }
                        )
                    inst["sync_info"]["on_wait"] = keep
                    changed = True
                new_insts.append(inst)
            bb["instructions"] = new_insts
    return orjson.dumps(d) if changed else raw


bass.Bass.to_json_bytes = _to_json_bytes_compat

F32 = mybir.dt.float32
BF16 = mybir.dt.bfloat16
ALU = mybir.AluOpType
ACTF = mybir.ActivationFunctionType
AXL = mybir.AxisListType

B, SQ, SKV, D = 4, 2048, 8192, 1024
FACTOR = 4
N_CORES = 8
W_PER_CORE = B * SQ // N_CORES  # 1024 windows per core
P = 128  # windows per tile = SBUF partitions
LN_EPS = 1e-5
HALF = 512  # PSUM bank free-dim (f32)


def build_bass(n_tiles=W_PER_CORE // P, repeats=1, ablate=(), dma=None, skew=True,
               ssq_stage="bn", qk_first=True, batch_io=True, diag_eng="dve"):
    """repeats>1 wraps the body in a For_i hardware loop (timing NEFFs);
    repeats=1 is the straight-line correctness/production NEFF.
    ablate: timing-only probes that skip work (results become wrong):
      'ssq' | 'bn' | 'qstat' | 'outmm' | 'qkadd' | 'smalls'
    dma: queue map (q, k, v, out), entries 'sp' | 'act'.
    skew: software-pipelined stage emission (False = naive per-tile order).
    """
    if dma is None:
        dma = ("sp", "sp", "act", "act")
    dma_q, dma_k, dma_v, dma_o = dma
    nc = bass.Bass()
    if batch_io:
        assert n_tiles % 4 == 0
        q_d = nc.declare_dram_parameter(
            "q", [n_tiles // 4, P, 4, D], BF16, isOutput=False
        )
        o_d = nc.declare_dram_parameter(
            "out", [n_tiles // 4, P, 4, D], BF16, isOutput=True
        )
    else:
        q_d = nc.declare_dram_parameter("q", [n_tiles * P, D], BF16, isOutput=False)
        o_d = nc.declare_dram_parameter("out", [n_tiles * P, D], BF16, isOutput=True)
    k_d = nc.declare_dram_parameter(
        "k", [n_tiles * P, FACTOR, D], BF16, isOutput=False
    )
    v_d = nc.declare_dram_parameter(
        "v", [n_tiles * P, FACTOR, D], BF16, isOutput=False
    )
    id_d = nc.declare_dram_parameter("ident", [P, P], BF16, isOutput=False)

    lp = nc.allow_low_precision(reason="bf16 data/accums: rel_err gate is 2e-2")
    lp.__enter__()

    with tile.TileContext(nc) as tc:
        with (
            tc.tile_pool(name="qp", bufs=4) as qp,
            tc.tile_pool(name="kp", bufs=4) as kp,
            tc.tile_pool(name="vp", bufs=4) as vp,
            tc.tile_pool(name="qkp", bufs=3) as qkp,
            tc.tile_pool(name="outp", bufs=2) as outp,
            tc.tile_pool(name="scratch", bufs=2) as scratch,
            tc.tile_pool(name="smalls", bufs=4) as sm,
            tc.tile_pool(name="const", bufs=1) as cp,
            tc.tile_pool(name="psum", bufs=3, space="PSUM") as pp,
        ):
            ident = cp.tile([P, P], BF16)
            nc.sync.dma_start(ident[:], id_d[:])

            from contextlib import nullcontext

            def eng(which):
                return {"sp": nc.sync, "act": nc.scalar}[which]

            st = {}  # t -> per-tile state dict

            def stage_load(t):
                rows = slice(t * P, (t + 1) * P)
                s = st.setdefault(t, {})
                if batch_io:
                    if t % 4 == 0:
                        qfat = qp.tile([P, 4, D], BF16, name="qfat", tag="q")
                        eng(dma_q).dma_start(qfat[:], q_d[t // 4])
                        st["qfat"] = qfat
                    s["q"] = st["qfat"][:, t % 4]
                else:
                    s["q"] = qp.tile([P, D], BF16, name="q", tag="q")
                    eng(dma_q).dma_start(s["q"][:], q_d[rows, :])
                s["k"] = kp.tile([P, FACTOR, D], BF16, name="k", tag="k")
                eng(dma_k).dma_start(s["k"][:], k_d[rows, :, :])
                s["v"] = vp.tile([P, FACTOR, D], BF16, name="v", tag="v")
                eng(dma_v).dma_start(s["v"][:], v_d[rows, :, :])

            def stage_bn(t):
                """Front: DVE q+k adds then k/v bn_stats; ACT q-stats then
                S-squares (consuming the adds as they land)."""
                s = st[t]
                q_sb, k_sb, v_sb = s["q"], s["k"], s["v"]
                sum_q = sm.tile([P, 1], F32)
                ssq_q = sm.tile([P, 1], F32)
                if "qstat" not in ablate:
                    dmpq = scratch.tile([P, D], BF16, tag="actdump")
                    nc.scalar.activation(
                        dmpq[:], q_sb[:], ACTF.Copy, accum_out=sum_q[:]
                    )
                    dmpq2 = scratch.tile([P, D], BF16, tag="actdump")
                    nc.scalar.activation(
                        dmpq2[:], q_sb[:], ACTF.Square, accum_out=ssq_q[:]
                    )
                else:
                    nc.gpsimd.memset(sum_q[:], 0.0)
                    nc.gpsimd.memset(ssq_q[:], 1.0)
                s["sum_q"], s["ssq_q"] = sum_q, ssq_q

                def emit_qk_adds():
                    qk = qkp.tile([P, FACTOR, D], BF16, name="qk")
                    if "qkadd" not in ablate:
                        for f in range(FACTOR):
                            nc.vector.tensor_tensor(
                                qk[:, f], k_sb[:, f], q_sb[:], ALU.add
                            )
                    s["qk"] = qk

                def emit_ssq():
                    S = sm.tile([P, FACTOR], F32, name="S")
                    if "ssq" not in ablate and "qkadd" not in ablate:
                        for f in range(FACTOR):
                            dmps = scratch.tile([P, D], BF16, tag="actdump")
                            nc.scalar.activation(
                                dmps[:], s["qk"][:, f], ACTF.Square,
                                accum_out=S[:, f : f + 1],
                            )
                    else:
                        nc.gpsimd.memset(S[:], 1.0)
                    s["S"] = S

                if qk_first:
                    emit_qk_adds()
                    if ssq_stage == "bn":
                        emit_ssq()
                s["emit_qk_adds"] = emit_qk_adds
                s["emit_ssq"] = emit_ssq

                bnst = sm.tile([P, 2, FACTOR, 2, 6], F32)
                stats = sm.tile([P, 2, FACTOR, 2], F32)  # (k/v, f, mean/var)
                if "bn" not in ablate:
                    for i, x_sb in ((0, k_sb), (1, v_sb)):
                        for f in range(FACTOR):
                            for ch in range(2):
                                nc.vector.bn_stats(
                                    bnst[:, i, f, ch],
                                    x_sb[:, f, ch * HALF : (ch + 1) * HALF],
                                )
                            nc.vector.bn_aggr(
                                stats[:, i, f],
                                bnst[:, i, f].rearrange("p c x -> p (c x)"),
                            )
                else:
                    nc.gpsimd.memset(stats[:], 0.5)
                s["stats"] = stats
                if not qk_first:
                    s["emit_qk_adds"]()
                    if ssq_stage == "bn":
                        s["emit_ssq"]()

            def stage_mid(t):
                """Small-tensor chain + diag builds (S/stats made last iter)."""
                s = st[t]
                if ssq_stage == "mid":
                    s["emit_ssq"]()
                stats = s["stats"]
                sum_q, ssq_q = s["sum_q"], s["ssq_q"]
                S = s["S"]
                mu_k = stats[:, 0, :, 0]
                mu_v = stats[:, 1, :, 0]

                mu_q = sm.tile([P, 1], F32)
                nc.gpsimd.tensor_scalar_mul(mu_q[:], sum_q[:], 1.0 / D)
                mmq = sm.tile([P, 1], F32)
                nc.gpsimd.tensor_mul(mmq[:], mu_q[:], mu_q[:])

                var_all = sm.tile([P, 9], F32)
                nc.gpsimd.tensor_scalar(
                    var_all[:, 0:1], ssq_q[:], 1.0 / D, mmq[:, 0:1],
                    ALU.mult, ALU.subtract,
                )
                nc.gpsimd.tensor_copy(var_all[:, 1:5], stats[:, 0, :, 1])
                nc.gpsimd.tensor_copy(var_all[:, 5:9], stats[:, 1, :, 1])
                veps = sm.tile([P, 9], F32)
                nc.gpsimd.tensor_scalar_add(veps[:], var_all[:], LN_EPS)
                rall = sm.tile([P, 9], F32)
                nc.vector.reciprocal(rall[:], veps[:])
                a_all = sm.tile([P, 9], F32)
                nc.scalar.sqrt(a_all[:], rall[:])
                aq = a_all[:, 0:1]
                ak = a_all[:, 1:5]
                av = a_all[:, 5:9]

                if "smalls" not in ablate:
                    mmk = sm.tile([P, FACTOR], F32)
                    nc.gpsimd.tensor_mul(mmk[:], mu_k, mu_k)
                    vpm = sm.tile([P, FACTOR], F32)
                    nc.gpsimd.tensor_tensor(
                        vpm[:], stats[:, 0, :, 1], mmk[:], ALU.add
                    )
                    # rdot = 0.5*(S - ssq_q) - 0.5*D*vpm
                    t_a = sm.tile([P, FACTOR], F32)
                    nc.gpsimd.tensor_scalar(
                        t_a[:], S[:], ssq_q[:, 0:1], 0.5, ALU.subtract, ALU.mult
                    )
                    rdot = sm.tile([P, FACTOR], F32)
                    nc.vector.scalar_tensor_tensor(
                        rdot[:], vpm[:], -0.5 * D, t_a[:], ALU.mult, ALU.add
                    )
                    # w_f = aq*ak_f*(rdot_f - D*muq*muk_f); c_f = w_f*av_f
                    t1 = sm.tile([P, FACTOR], F32)
                    nc.gpsimd.tensor_scalar(
                        t1[:], mu_k, mu_q[:, 0:1], None, ALU.mult
                    )
                    t2 = sm.tile([P, FACTOR], F32)
                    nc.vector.scalar_tensor_tensor(
                        t2[:], t1[:], -float(D), rdot[:], ALU.mult, ALU.add
                    )
                    u = sm.tile([P, FACTOR], F32)
                    nc.gpsimd.tensor_scalar(u[:], ak, aq, None, ALU.mult)
                    w = sm.tile([P, FACTOR], F32)
                    nc.gpsimd.tensor_mul(w[:], t2[:], u[:])
                    c = sm.tile([P, FACTOR], F32)
                    nc.gpsimd.tensor_mul(c[:], w[:], av)
                    e = sm.tile([P, FACTOR], F32)
                    nc.gpsimd.tensor_mul(e[:], c[:], mu_v)
                    neg_d = sm.tile([P, 1], F32)
                    nc.vector.tensor_reduce(
                        neg_d[:], e[:], AXL.X, ALU.add, negate=True
                    )
                else:
                    c = sm.tile([P, FACTOR], F32)
                    nc.gpsimd.tensor_scalar_mul(c[:], S[:], 1.0)
                    neg_d = sm.tile([P, 1], F32)
                    nc.gpsimd.memset(neg_d[:], 0.0)
                s["neg_d"] = neg_d

                diags = []
                diag_e = nc.vector if diag_eng == "dve" else nc.gpsimd
                for f in range(FACTOR):
                    dg = sm.tile([P, P], BF16, tag=f"diag{f}")
                    diag_e.tensor_scalar_mul(dg[:], ident[:], c[:, f : f + 1])
                    diags.append(dg)
                s["diags"] = diags

            def stage_out(t):
                """PE accumulation + ACT writeback + out DMA."""
                s = st.pop(t)
                q_sb, v_sb, diags = s["q"], s["v"], s["diags"]
                psum_t = pp.tile([P, 2, HALF], F32)
                if "outmm" not in ablate:
                    for h in range(2):
                        nc.tensor.matmul(
                            psum_t[:, h],
                            ident[:],
                            q_sb[:, h * HALF : (h + 1) * HALF],
                            start=True,
                            stop=False,
                        )
                    for f in range(FACTOR):
                        for h in range(2):
                            nc.tensor.matmul(
                                psum_t[:, h],
                                diags[f][:],
                                v_sb[:, f, h * HALF : (h + 1) * HALF],
                                start=False,
                                stop=(f == FACTOR - 1),
                            )
                else:
                    for h in range(2):
                        nc.tensor.matmul(
                            psum_t[:, h],
                            diags[0][:],
                            v_sb[:, 0, h * HALF : (h + 1) * HALF],
                            start=True,
                            stop=True,
                        )
                if batch_io:
                    if t % 4 == 0:
                        st["outfat"] = outp.tile([P, 4, D], BF16, name="outfat")
                    out_sb = st["outfat"][:, t % 4]
                else:
                    out_sb = outp.tile([P, D], BF16, name="out_sb")
                nc.scalar.activation(
                    out_sb[:],
                    psum_t[:].rearrange("p c x -> p (c x)"),
                    ACTF.Identity,
                    bias=s["neg_d"][:],
                )
                if batch_io:
                    if t % 4 == 3:
                        eng(dma_o).dma_start(o_d[t // 4], st["outfat"][:])
                else:
                    rows = slice(t * P, (t + 1) * P)
                    eng(dma_o).dma_start(o_d[rows, :], out_sb[:])

            loop_ctx = tc.For_i(0, repeats, 1) if repeats > 1 else nullcontext()
            with loop_ctx:
                if skew:
                    for i in range(n_tiles + 3):
                        if i < n_tiles:
                            stage_load(i)
                        if 0 <= i - 1 < n_tiles:
                            stage_bn(i - 1)
                        if 0 <= i - 2 < n_tiles:
                            stage_mid(i - 2)
                        if 0 <= i - 3 < n_tiles:
                            stage_out(i - 3)
                else:
                    for t in range(n_tiles):
                        stage_load(t)
                        stage_bn(t)
                        stage_mid(t)
                        stage_out(t)
    return nc


def make_in_map(q_core, k_core, v_core, batch_io=True):
    """Host-side per-core input prep shared by run()/test/sim: cast to bf16.
    batch_io permutes q to [T/4, P, 4, D] so 4 tiles load as one fat DMA."""
    import ml_dtypes

    bf = ml_dtypes.bfloat16
    q = np.asarray(q_core, dtype=np.float32)
    if batch_io:
        T = q.shape[0] // P
        q = q.reshape(T // 4, 4, P, D).transpose(0, 2, 1, 3)
    return {
        "q": np.ascontiguousarray(q).astype(bf),
        "k": np.ascontiguousarray(np.asarray(k_core, dtype=np.float32)).astype(bf),
        "v": np.ascontiguousarray(np.asarray(v_core, dtype=np.float32)).astype(bf),
        "ident": np.eye(P, dtype=np.float32).astype(bf),
    }


def unpermute_out(out_core, batch_io=True):
    """Invert the batch_io out permutation: [T/4, P, 4, D] -> [T*P, D]."""
    if not batch_io:
        return np.asarray(out_core, dtype=np.float32)
    o = np.asarray(out_core, dtype=np.float32)
    g = o.shape[0]
    return o.transpose(0, 2, 1, 3).reshape(g * 4 * P, D)


_NC_CACHE = None


def _get_nc():
    global _NC_CACHE
    if _NC_CACHE is None:
        _NC_CACHE = build_bass()
    return _NC_CACHE


def _numpy_reference(query, key, value, ln_w, ln_b):
    def ln(x):
        mu = x.mean(-1, keepdims=True)
        var = ((x - mu) ** 2).mean(-1, keepdims=True)
        return (x - mu) / np.sqrt(var + LN_EPS) * ln_w + ln_b

    qn, kn, vn = ln(query), ln(key), ln(value)
    b, s, d = key.shape
    k_win = kn.reshape(b, s // FACTOR, FACTOR, d)
    wts = np.einsum("bsd,bsfd->bsf", qn, k_win).reshape(b, s)
    attn = (wts[:, :, None] * vn).reshape(b, s // FACTOR, FACTOR, d).sum(axis=2)
    return (query + attn).astype(np.float32)


def run(inputs, trace=False):
    """Returns (full_output, BassKernelResults-or-None)."""
    query = np.asarray(inputs["query"], dtype=np.float32)
    key = np.asarray(inputs["key"], dtype=np.float32)
    value = np.asarray(inputs["value"], dtype=np.float32)
    ln_w = np.asarray(inputs["ln_weight"], dtype=np.float32)
    ln_b = np.asarray(inputs["ln_bias"], dtype=np.float32)

    if not (np.all(ln_w == 1.0) and np.all(ln_b == 0.0)):
        # General-path fallback (setup_inputs always produces ones/zeros).
        return _numpy_reference(query, key, value, ln_w, ln_b), None

    sq_h = SQ // 2  # 1024 query rows per core
    skv_h = SKV // 2  # 4096 kv rows per core
    in_maps = []
    for cidx in range(N_CORES):
        bi, h = divmod(cidx, 2)
        in_maps.append(
            make_in_map(
                query[bi, h * sq_h : (h + 1) * sq_h],
                key[bi, h * skv_h : (h + 1) * skv_h].reshape(W_PER_CORE, FACTOR, D),
                value[bi, h * skv_h : (h + 1) * skv_h].reshape(W_PER_CORE, FACTOR, D),
            )
        )

    res = run_bass_kernel_spmd(
        _get_nc(), in_maps, core_ids=list(range(N_CORES)), trace=trace
    )
    out = np.empty((B, SQ, D), dtype=np.float32)
    for cidx in range(N_CORES):
        bi, h = divmod(cidx, 2)
        out[bi, h * sq_h : (h + 1) * sq_h] = unpermute_out(
            res.results[cidx]["out"]
        )
    return out, res


def kernel(**inputs) -> np.ndarray:
    out, _ = run(inputs)
    return out


# revision 31
# speedup vs baseline: 1.1649x; 1.0448x over previous
"""Bass/Tile Trainium2 kernel for nn_AttentionSampling.

Problem: out = q + attention_downsampling(LN(q), LN(k), LN(v), factor=4)
  B=4, Sq=2048, Skv=8192, D=1024. Per query token s:
    w_f   = dot(LN(q)[s], LN(k)[4s+f])          f in 0..3  (no softmax)
    out[s] = q[s] + sum_f w_f * LN(v)[4s+f]

Key algebraic folding (valid for ln_weight==1, ln_bias==0, which is what
setup_inputs produces; a numpy fallback handles the general case):
    dot(LN(q), LN(k)) = aq*ak*(q.k - D*muq*muk)      a = rsqrt(var+eps)
    sum_f w_f*LN(v_f) = sum_f c_f*v_f - (sum_f c_f*muv_f)*ones,  c_f = w_f*av_f
so no normalized tensor is ever materialized: only raw dots + per-token stats.

v3: software-pipelined emission. HW probing showed the kernel is dependency-
chain bound, not engine-throughput bound: ACT/DVE/Pool have strict in-order
FIFOs, so tile t's late-chain instructions (writeback, rdot) emitted before
tile t+1's early work (bn_stats, q stats) block it -> near-serial per-tile
chains. Fix: emit stages skewed (loads t, DVE stats t-1, mid-chain t-2,
out-path t-3) so every FIFO sees only near-ready instructions.

Sharding: 8 cores = batch (4) x query-half (2). Each core owns 1024 windows:
q[1024,1024], k/v[1024,4,1024] (window-major view), out[1024,1024].
"""

import numpy as np


def _ensure_concourse():
    try:
        import concourse.bass  # noqa: F401
    except ImportError:
        import sys

        for p in ("/opt/trn_rl_repo", "/root/.axon_site/_ro/trn_rl_repo"):
            if p not in sys.path:
                sys.path.insert(0, p)


_ensure_concourse()

import concourse.bass as bass  # noqa: E402
import concourse.tile as tile  # noqa: E402
from concourse import mybir  # noqa: E402
from concourse.bass_utils import run_bass_kernel_spmd  # noqa: E402

# ---------------------------------------------------------------------------
# Walrus-compatibility shims.
#
# The walrus in this container rejects two things Tile's end-of-context tail
# emits: (a) the final Drain carrying >2 sem waits ("Too many sync wait
# commands"), and (b) EVENT_SEMAPHORE_RANGE_CLEAR ("ISA wrong length").
# Replace the tail with per-semaphore EventSemaphore instructions that wait
# for each sem's final value, then the normal all-engine barrier. A JSON-level
# pass additionally splits any instruction carrying more than MAX_WAITS sem
# waits into EventSemaphore wait carriers.
# ---------------------------------------------------------------------------

_MAX_WAITS = 1


def _patched_drain_and_barrier(self, tick_clock, wait_clock):
    nc = self.nc
    gc = tick_clock.global_clock
    sems = self.sems.allocated()  # proc idx -> SemaphoreHandle
    for proc in sorted(sems):
        h = sems[proc]
        if "DMA" not in h.name:
            continue  # engine sems are implied by stream completion
        final = int(gc[proc]) * 16
        if final > 0:
            nc.gpsimd.wait_ge(h, final)
    nc.all_engine_barrier()
    popped = nc._tile_sem_poison_stack.pop()
    assert popped is self._sem_poison


tile.TileContext._drain_and_barrier = _patched_drain_and_barrier

_orig_to_json_bytes = bass.Bass.to_json_bytes


def _to_json_bytes_compat(self):
    import orjson

    raw = _orig_to_json_bytes(self)
    d = orjson.loads(raw)
    changed = False
    for fn in d.get("functions", []):
        blocks = fn.get("basic_blocks") or fn.get("blocks") or []
        for bb in blocks:
            insts = bb.get("instructions", [])
            new_insts = []
            for inst in insts:
                waits = (inst.get("sync_info") or {}).get("on_wait") or []
                if len(waits) > _MAX_WAITS:
                    keep = waits[-_MAX_WAITS:]
                    excess = waits[:-_MAX_WAITS]
                    for i, wt in enumerate(excess):
                        new_insts.append(
                            {
                                "name": f"{inst['name']}_wsplit{i}",
                                "opcode": "EventSemaphore",
                                "engine": inst["engine"],
                                "ins": [],
                                "outs": [],
                                "debug": inst.get("debug"),
                                "sync_info": {"on_update": [], "on_wait": [wt]},
                            }
                        )
                    inst["sync_info"]["on_wait"] = keep
                    changed = True
                new_insts.append(inst)
            bb["instructions"] = new_insts
    return orjson.dumps(d) if changed else raw


bass.Bass.to_json_bytes = _to_json_bytes_compat

F32 = mybir.dt.float32
BF16 = mybir.dt.bfloat16
ALU = mybir.AluOpType
ACTF = mybir.ActivationFunctionType
AXL = mybir.AxisListType

B, SQ, SKV, D = 4, 2048, 8192, 1024
FACTOR = 4
N_CORES = 8
W_PER_CORE = B * SQ // N_CORES  # 1024 windows per core
P = 128  # windows per tile = SBUF partitions
LN_EPS = 1e-5
HALF = 512  # PSUM bank free-dim (f32)


def build_bass(n_tiles=W_PER_CORE // P, repeats=1, ablate=(), dma=None, skew=True,
               ssq_stage="mid", qk_first=False, batch_io=True, diag_eng="dve",
               dots="strick", wb="act", qstat_eng="act", psum_bufs=3,
               load_bufs=4, vstat_act=1, smalls_eng="pool"):
    """repeats>1 wraps the body in a For_i hardware loop (timing NEFFs);
    repeats=1 is the straight-line correctness/production NEFF.
    ablate: timing-only probes that skip work (results become wrong):
      'ssq' | 'bn' | 'qstat' | 'outmm' | 'qkadd' | 'smalls'
    dma: queue map (q, k, v, out), entries 'sp' | 'act'.
    skew: software-pipelined stage emission (False = naive per-tile order).
    """
    if dma is None:
        dma = ("sp", "sp", "act", "act")
    dma_q, dma_k, dma_v, dma_o = dma
    nc = bass.Bass()
    if batch_io:
        assert n_tiles % 4 == 0
        q_d = nc.declare_dram_parameter(
            "q", [n_tiles // 4, P, 4, D], BF16, isOutput=False
        )
        o_d = nc.declare_dram_parameter(
            "out", [n_tiles // 4, P, 4, D], BF16, isOutput=True
        )
    else:
        q_d = nc.declare_dram_parameter("q", [n_tiles * P, D], BF16, isOutput=False)
        o_d = nc.declare_dram_parameter("out", [n_tiles * P, D], BF16, isOutput=True)
    k_d = nc.declare_dram_parameter(
        "k", [n_tiles * P, FACTOR, D], BF16, isOutput=False
    )
    v_d = nc.declare_dram_parameter(
        "v", [n_tiles * P, FACTOR, D], BF16, isOutput=False
    )
    id_d = nc.declare_dram_parameter("ident", [P, P], BF16, isOutput=False)

    lp = nc.allow_low_precision(reason="bf16 data/accums: rel_err gate is 2e-2")
    lp.__enter__()

    with tile.TileContext(nc) as tc:
        with (
            tc.tile_pool(name="qp", bufs=load_bufs) as qp,
            tc.tile_pool(name="kp", bufs=load_bufs) as kp,
            tc.tile_pool(name="vp", bufs=load_bufs) as vp,
            tc.tile_pool(name="qkp", bufs=3) as qkp,
            tc.tile_pool(name="outp", bufs=2) as outp,
            tc.tile_pool(name="scratch", bufs=2) as scratch,
            tc.tile_pool(name="smalls", bufs=4) as sm,
            tc.tile_pool(name="const", bufs=1) as cp,
            tc.tile_pool(name="psum", bufs=psum_bufs, space="PSUM") as pp,
            tc.tile_pool(name="qkpsum", bufs=2, space="PSUM") as qkpp,
        ):
            ident = cp.tile([P, P], BF16)
            nc.sync.dma_start(ident[:], id_d[:])

            from contextlib import nullcontext

            def eng(which):
                return {"sp": nc.sync, "act": nc.scalar}[which]

            st = {}  # t -> per-tile state dict

            def stage_load(t):
                rows = slice(t * P, (t + 1) * P)
                s = st.setdefault(t, {})
                if batch_io:
                    if t % 4 == 0:
                        qfat = qp.tile([P, 4, D], BF16, name="qfat", tag="q")
                        eng(dma_q).dma_start(qfat[:], q_d[t // 4])
                        st["qfat"] = qfat
                    s["q"] = st["qfat"][:, t % 4]
                else:
                    s["q"] = qp.tile([P, D], BF16, name="q", tag="q")
                    eng(dma_q).dma_start(s["q"][:], q_d[rows, :])
                s["k"] = kp.tile([P, FACTOR, D], BF16, name="k", tag="k")
                eng(dma_k).dma_start(s["k"][:], k_d[rows, :, :])
                s["v"] = vp.tile([P, FACTOR, D], BF16, name="v", tag="v")
                eng(dma_v).dma_start(s["v"][:], v_d[rows, :, :])

            def stage_bn(t):
                """Front: DVE q+k adds then k/v bn_stats; ACT q-stats then
                S-squares (consuming the adds as they land)."""
                s = st[t]
                q_sb, k_sb, v_sb = s["q"], s["k"], s["v"]
                sum_q = sm.tile([P, 1], F32)
                ssq_q = sm.tile([P, 1], F32)
                if "qstat" in ablate:
                    nc.gpsimd.memset(sum_q[:], 0.0)
                    nc.gpsimd.memset(ssq_q[:], 1.0)
                elif qstat_eng == "dve":
                    qbn = sm.tile([P, 2, 6], F32, name="qbn")
                    for ch in range(2):
                        nc.vector.bn_stats(
                            qbn[:, ch], q_sb[:, ch * HALF : (ch + 1) * HALF]
                        )
                    qstats = sm.tile([P, 2], F32, name="qstats")
                    nc.vector.bn_aggr(
                        qstats[:], qbn[:].rearrange("p c x -> p (c x)")
                    )
                    # sum_q = mu*D ; ssq_q = (var + mu^2)*D
                    nc.gpsimd.tensor_scalar_mul(
                        sum_q[:], qstats[:, 0:1], float(D)
                    )
                    qmm = sm.tile([P, 1], F32, name="qmm")
                    nc.gpsimd.tensor_mul(qmm[:], qstats[:, 0:1], qstats[:, 0:1])
                    nc.gpsimd.tensor_scalar(
                        ssq_q[:], qstats[:, 1:2], qmm[:, 0:1], float(D),
                        ALU.add, ALU.mult,
                    )
                else:
                    dmpq = scratch.tile([P, D], BF16, tag="actdump")
                    nc.scalar.activation(
                        dmpq[:], q_sb[:], ACTF.Copy, accum_out=sum_q[:]
                    )
                    dmpq2 = scratch.tile([P, D], BF16, tag="actdump")
                    nc.scalar.activation(
                        dmpq2[:], q_sb[:], ACTF.Square, accum_out=ssq_q[:]
                    )
                s["sum_q"], s["ssq_q"] = sum_q, ssq_q

                def emit_qk_adds():
                    if dots in ("ttr", "pe"):
                        return
                    qk = qkp.tile([P, FACTOR, D], BF16, name="qk")
                    if "qkadd" not in ablate:
                        for f in range(FACTOR):
                            nc.vector.tensor_tensor(
                                qk[:, f], k_sb[:, f], q_sb[:], ALU.add
                            )
                    s["qk"] = qk

                def emit_ttr_dots():
                    rdot = sm.tile([P, FACTOR], F32, name="rdot_ttr")
                    if "ssq" not in ablate:
                        for f in range(FACTOR):
                            dmps = qkp.tile([P, D], BF16, tag="ttrdump")
                            nc.vector.tensor_tensor_reduce(
                                dmps[:], k_sb[:, f], q_sb[:], 1.0, 0.0,
                                ALU.mult, ALU.add,
                                accum_out=rdot[:, f : f + 1],
                            )
                    else:
                        nc.gpsimd.memset(rdot[:], 1.0)
                    s["rdot"] = rdot

                def emit_ssq():
                    if dots == "ttr":
                        emit_ttr_dots()
                        return
                    if dots == "pe":
                        S_h = sm.tile([P, 2, FACTOR], F32, name="S_h")
                        for f in range(FACTOR):
                            for h in range(2):
                                qk_ps = qkpp.tile([P, HALF], F32, tag="qkps")
                                nc.tensor.matmul(
                                    qk_ps[:],
                                    ident[:],
                                    q_sb[:, h * HALF : (h + 1) * HALF],
                                    start=True,
                                    stop=False,
                                )
                                nc.tensor.matmul(
                                    qk_ps[:],
                                    ident[:],
                                    k_sb[:, f, h * HALF : (h + 1) * HALF],
                                    start=False,
                                    stop=True,
                                )
                                dmh = scratch.tile([P, HALF], BF16, tag="acthalf")
                                nc.scalar.activation(
                                    dmh[:],
                                    qk_ps[:],
                                    ACTF.Square,
                                    accum_out=S_h[:, h, f : f + 1],
                                )
                        S = sm.tile([P, FACTOR], F32, name="S")
                        nc.vector.tensor_tensor(
                            S[:], S_h[:, 0], S_h[:, 1], ALU.add
                        )
                        s["S"] = S
                        return
                    S = sm.tile([P, FACTOR], F32, name="S")
                    if "ssq" not in ablate and "qkadd" not in ablate:
                        for f in range(FACTOR):
                            dmps = scratch.tile([P, D], BF16, tag="actdump")
                            nc.scalar.activation(
                                dmps[:], s["qk"][:, f], ACTF.Square,
                                accum_out=S[:, f : f + 1],
                            )
                    else:
                        nc.gpsimd.memset(S[:], 1.0)
                    s["S"] = S

                if qk_first:
                    emit_qk_adds()
                    if ssq_stage == "bn":
                        emit_ssq()
                s["emit_qk_adds"] = emit_qk_adds
                s["emit_ssq"] = emit_ssq

                bnst = sm.tile([P, 2, FACTOR, 2, 6], F32)
                stats = sm.tile([P, 2, FACTOR, 2], F32)  # (k/v, f, mean/var)
                if "bn" not in ablate:
                    vsums = sm.tile([P, 2, FACTOR], F32, name="vsums")
                    for i, x_sb in ((0, k_sb), (1, v_sb)):
                        for f in range(FACTOR):
                            if i == 1 and f >= FACTOR - vstat_act:
                                # v-stats via ACT 2-pass accumulate
                                dva = scratch.tile([P, D], BF16, tag="actdump")
                                nc.scalar.activation(
                                    dva[:], x_sb[:, f], ACTF.Copy,
                                    accum_out=vsums[:, 0, f : f + 1],
                                )
                                dvb = scratch.tile([P, D], BF16, tag="actdump")
                                nc.scalar.activation(
                                    dvb[:], x_sb[:, f], ACTF.Square,
                                    accum_out=vsums[:, 1, f : f + 1],
                                )
                                # mean = sum/D; var = ssq/D - mean^2
                                nc.gpsimd.tensor_scalar_mul(
                                    stats[:, 1, f, 0:1],
                                    vsums[:, 0, f : f + 1],
                                    1.0 / D,
                                )
                                vm2 = sm.tile([P, 1], F32, tag=f"vm2_{f}")
                                nc.gpsimd.tensor_mul(
                                    vm2[:],
                                    stats[:, 1, f, 0:1],
                                    stats[:, 1, f, 0:1],
                                )
                                nc.gpsimd.tensor_scalar(
                                    stats[:, 1, f, 1:2],
                                    vsums[:, 1, f : f + 1],
                                    1.0 / D,
                                    vm2[:, 0:1],
                                    ALU.mult,
                                    ALU.subtract,
                                )
                                continue
                            for ch in range(2):
                                nc.vector.bn_stats(
                                    bnst[:, i, f, ch],
                                    x_sb[:, f, ch * HALF : (ch + 1) * HALF],
                                )
                            nc.vector.bn_aggr(
                                stats[:, i, f],
                                bnst[:, i, f].rearrange("p c x -> p (c x)"),
                            )
                else:
                    nc.gpsimd.memset(stats[:], 0.5)
                s["stats"] = stats
                if not qk_first:
                    s["emit_qk_adds"]()
                    if ssq_stage == "bn":
                        s["emit_ssq"]()

            def stage_mid(t):
                """Small-tensor chain + diag builds (S/stats made last iter)."""
                s = st[t]
                if ssq_stage == "mid":
                    s["emit_ssq"]()
                stats = s["stats"]
                sum_q, ssq_q = s["sum_q"], s["ssq_q"]
                S = s.get("S")
                mu_k = stats[:, 0, :, 0]
                mu_v = stats[:, 1, :, 0]

                se = nc.vector if smalls_eng == "dve" else nc.gpsimd
                mu_q = sm.tile([P, 1], F32)
                se.tensor_scalar_mul(mu_q[:], sum_q[:], 1.0 / D)
                mmq = sm.tile([P, 1], F32)
                se.tensor_mul(mmq[:], mu_q[:], mu_q[:])

                var_all = sm.tile([P, 9], F32)
                se.tensor_scalar(
                    var_all[:, 0:1], ssq_q[:], 1.0 / D, mmq[:, 0:1],
                    ALU.mult, ALU.subtract,
                )
                se.tensor_copy(var_all[:, 1:5], stats[:, 0, :, 1])
                se.tensor_copy(var_all[:, 5:9], stats[:, 1, :, 1])
                veps = sm.tile([P, 9], F32)
                se.tensor_scalar_add(veps[:], var_all[:], LN_EPS)
                rall = sm.tile([P, 9], F32)
                nc.vector.reciprocal(rall[:], veps[:])
                a_all = sm.tile([P, 9], F32)
                nc.scalar.sqrt(a_all[:], rall[:])
                aq = a_all[:, 0:1]
                ak = a_all[:, 1:5]
                av = a_all[:, 5:9]

                if "smalls" not in ablate:
                    if dots == "ttr":
                        rdot = s["rdot"]
                    else:
                        mmk = sm.tile([P, FACTOR], F32)
                        se.tensor_mul(mmk[:], mu_k, mu_k)
                        vpm = sm.tile([P, FACTOR], F32)
                        se.tensor_tensor(
                            vpm[:], stats[:, 0, :, 1], mmk[:], ALU.add
                        )
                        # rdot = 0.5*(S - ssq_q) - 0.5*D*vpm
                        t_a = sm.tile([P, FACTOR], F32)
                        se.tensor_scalar(
                            t_a[:], S[:], ssq_q[:, 0:1], 0.5, ALU.subtract, ALU.mult
                        )
                        rdot = sm.tile([P, FACTOR], F32)
                        nc.vector.scalar_tensor_tensor(
                            rdot[:], vpm[:], -0.5 * D, t_a[:], ALU.mult, ALU.add
                        )
                    # w_f = aq*ak_f*(rdot_f - D*muq*muk_f); c_f = w_f*av_f
                    t1 = sm.tile([P, FACTOR], F32)
                    se.tensor_scalar(
                        t1[:], mu_k, mu_q[:, 0:1], None, ALU.mult
                    )
                    t2 = sm.tile([P, FACTOR], F32)
                    nc.vector.scalar_tensor_tensor(
                        t2[:], t1[:], -float(D), rdot[:], ALU.mult, ALU.add
                    )
                    u = sm.tile([P, FACTOR], F32)
                    se.tensor_scalar(u[:], ak, aq, None, ALU.mult)
                    w = sm.tile([P, FACTOR], F32)
                    se.tensor_mul(w[:], t2[:], u[:])
                    c = sm.tile([P, FACTOR], F32)
                    se.tensor_mul(c[:], w[:], av)
                    e = sm.tile([P, FACTOR], F32)
                    se.tensor_mul(e[:], c[:], mu_v)
                    neg_d = sm.tile([P, 1], F32)
                    nc.vector.tensor_reduce(
                        neg_d[:], e[:], AXL.X, ALU.add, negate=True
                    )
                else:
                    c = sm.tile([P, FACTOR], F32)
                    nc.gpsimd.tensor_scalar_mul(c[:], S[:], 1.0)
                    neg_d = sm.tile([P, 1], F32)
                    nc.gpsimd.memset(neg_d[:], 0.0)
                s["neg_d"] = neg_d

                diags = []
                diag_e = nc.vector if diag_eng == "dve" else nc.gpsimd
                for f in range(FACTOR):
                    dg = sm.tile([P, P], BF16, tag=f"diag{f}")
                    diag_e.tensor_scalar_mul(dg[:], ident[:], c[:, f : f + 1])
                    diags.append(dg)
                s["diags"] = diags

            def stage_out(t):
                """PE accumulation + ACT writeback + out DMA."""
                s = st.pop(t)
                q_sb, v_sb, diags = s["q"], s["v"], s["diags"]
                psum_t = pp.tile([P, 2, HALF], F32)
                if "outmm" not in ablate:
                    for h in range(2):
                        if wb != "dve":
                            nc.tensor.matmul(
                                psum_t[:, h],
                                ident[:],
                                q_sb[:, h * HALF : (h + 1) * HALF],
                                start=True,
                                stop=False,
                            )
                    for f in range(FACTOR):
                        for h in range(2):
                            nc.tensor.matmul(
                                psum_t[:, h],
                                diags[f][:],
                                v_sb[:, f, h * HALF : (h + 1) * HALF],
                                start=(wb == "dve" and f == 0),
                                stop=(f == FACTOR - 1),
                            )
                else:
                    for h in range(2):
                        nc.tensor.matmul(
                            psum_t[:, h],
                            diags[0][:],
                            v_sb[:, 0, h * HALF : (h + 1) * HALF],
                            start=True,
                            stop=True,
                        )
                if batch_io:
                    if t % 4 == 0:
                        st["outfat"] = outp.tile([P, 4, D], BF16, name="outfat")
                    out_sb = st["outfat"][:, t % 4]
                else:
                    out_sb = outp.tile([P, D], BF16, name="out_sb")
                if wb == "dve":
                    # out = (neg_d + psum_attn) + q in one DVE STT
                    nc.vector.scalar_tensor_tensor(
                        out_sb[:],
                        psum_t[:].rearrange("p c x -> p (c x)"),
                        s["neg_d"][:, 0:1],
                        q_sb[:],
                        ALU.add,
                        ALU.add,
                    )
                else:
                    nc.scalar.activation(
                        out_sb[:],
                        psum_t[:].rearrange("p c x -> p (c x)"),
                        ACTF.Identity,
                        bias=s["neg_d"][:],
                    )
                if batch_io:
                    if t % 4 == 3:
                        eng(dma_o).dma_start(o_d[t // 4], st["outfat"][:])
                else:
                    rows = slice(t * P, (t + 1) * P)
                    eng(dma_o).dma_start(o_d[rows, :], out_sb[:])

            loop_ctx = tc.For_i(0, repeats, 1) if repeats > 1 else nullcontext()
            with loop_ctx:
                if skew:
                    for i in range(n_tiles + 3):
                        if i < n_tiles:
                            stage_load(i)
                        if 0 <= i - 1 < n_tiles:
                            stage_bn(i - 1)
                        if 0 <= i - 2 < n_tiles:
                            stage_mid(i - 2)
                        if 0 <= i - 3 < n_tiles:
                            stage_out(i - 3)
                else:
                    for t in range(n_tiles):
                        stage_load(t)
                        stage_bn(t)
                        stage_mid(t)
                        stage_out(t)
    return nc


def make_in_map(q_core, k_core, v_core, batch_io=True):
    """Host-side per-core input prep shared by run()/test/sim: cast to bf16.
    batch_io permutes q to [T/4, P, 4, D] so 4 tiles load as one fat DMA."""
    import ml_dtypes

    bf = ml_dtypes.bfloat16
    q = np.asarray(q_core, dtype=np.float32)
    if batch_io:
        T = q.shape[0] // P
        q = q.reshape(T // 4, 4, P, D).transpose(0, 2, 1, 3)
    return {
        "q": np.ascontiguousarray(q).astype(bf),
        "k": np.ascontiguousarray(np.asarray(k_core, dtype=np.float32)).astype(bf),
        "v": np.ascontiguousarray(np.asarray(v_core, dtype=np.float32)).astype(bf),
        "ident": np.eye(P, dtype=np.float32).astype(bf),
    }


def unpermute_out(out_core, batch_io=True):
    """Invert the batch_io out permutation: [T/4, P, 4, D] -> [T*P, D]."""
    if not batch_io:
        return np.asarray(out_core, dtype=np.float32)
    o = np.asarray(out_core, dtype=np.float32)
    g = o.shape[0]
    return o.transpose(0, 2, 1, 3).reshape(g * 4 * P, D)


_NC_CACHE = None


def _get_nc():
    global _NC_CACHE
    if _NC_CACHE is None:
        _NC_CACHE = build_bass()
    return _NC_CACHE


def _numpy_reference(query, key, value, ln_w, ln_b):
    def ln(x):
        mu = x.mean(-1, keepdims=True)
        var = ((x - mu) ** 2).mean(-1, keepdims=True)
        return (x - mu) / np.sqrt(var + LN_EPS) * ln_w + ln_b

    qn, kn, vn = ln(query), ln(key), ln(value)
    b, s, d = key.shape
    k_win = kn.reshape(b, s // FACTOR, FACTOR, d)
    wts = np.einsum("bsd,bsfd->bsf", qn, k_win).reshape(b, s)
    attn = (wts[:, :, None] * vn).reshape(b, s // FACTOR, FACTOR, d).sum(axis=2)
    return (query + attn).astype(np.float32)


def run(inputs, trace=False):
    """Returns (full_output, BassKernelResults-or-None)."""
    query = np.asarray(inputs["query"], dtype=np.float32)
    key = np.asarray(inputs["key"], dtype=np.float32)
    value = np.asarray(inputs["value"], dtype=np.float32)
    ln_w = np.asarray(inputs["ln_weight"], dtype=np.float32)
    ln_b = np.asarray(inputs["ln_bias"], dtype=np.float32)

    if not (np.all(ln_w == 1.0) and np.all(ln_b == 0.0)):
        # General-path fallback (setup_inputs always produces ones/zeros).
        return _numpy_reference(query, key, value, ln_w, ln_b), None

    sq_h = SQ // 2  # 1024 query rows per core
    skv_h = SKV // 2  # 4096 kv rows per core
    in_maps = []
    for cidx in range(N_CORES):
        bi, h = divmod(cidx, 2)
        in_maps.append(
            make_in_map(
                query[bi, h * sq_h : (h + 1) * sq_h],
                key[bi, h * skv_h : (h + 1) * skv_h].reshape(W_PER_CORE, FACTOR, D),
                value[bi, h * skv_h : (h + 1) * skv_h].reshape(W_PER_CORE, FACTOR, D),
            )
        )

    res = run_bass_kernel_spmd(
        _get_nc(), in_maps, core_ids=list(range(N_CORES)), trace=trace
    )
    out = np.empty((B, SQ, D), dtype=np.float32)
    for cidx in range(N_CORES):
        bi, h = divmod(cidx, 2)
        out[bi, h * sq_h : (h + 1) * sq_h] = unpermute_out(
            res.results[cidx]["out"]
        )
    return out, res


def kernel(**inputs) -> np.ndarray:
    out, _ = run(inputs)
    return out


# revision 32
# speedup vs baseline: 1.1983x; 1.0286x over previous
"""Bass/Tile Trainium2 kernel for nn_AttentionSampling.

Problem: out = q + attention_downsampling(LN(q), LN(k), LN(v), factor=4)
  B=4, Sq=2048, Skv=8192, D=1024. Per query token s:
    w_f   = dot(LN(q)[s], LN(k)[4s+f])          f in 0..3  (no softmax)
    out[s] = q[s] + sum_f w_f * LN(v)[4s+f]

Key algebraic folding (valid for ln_weight==1, ln_bias==0, which is what
setup_inputs produces; a numpy fallback handles the general case):
    dot(LN(q), LN(k)) = aq*ak*(q.k - D*muq*muk)      a = rsqrt(var+eps)
    sum_f w_f*LN(v_f) = sum_f c_f*v_f - (sum_f c_f*muv_f)*ones,  c_f = w_f*av_f
so no normalized tensor is ever materialized: only raw dots + per-token stats.

v3: software-pipelined emission. HW probing showed the kernel is dependency-
chain bound, not engine-throughput bound: ACT/DVE/Pool have strict in-order
FIFOs, so tile t's late-chain instructions (writeback, rdot) emitted before
tile t+1's early work (bn_stats, q stats) block it -> near-serial per-tile
chains. Fix: emit stages skewed (loads t, DVE stats t-1, mid-chain t-2,
out-path t-3) so every FIFO sees only near-ready instructions.

Sharding: 8 cores = batch (4) x query-half (2). Each core owns 1024 windows:
q[1024,1024], k/v[1024,4,1024] (window-major view), out[1024,1024].
"""

import numpy as np


def _ensure_concourse():
    try:
        import concourse.bass  # noqa: F401
    except ImportError:
        import sys

        for p in ("/opt/trn_rl_repo", "/root/.axon_site/_ro/trn_rl_repo"):
            if p not in sys.path:
                sys.path.insert(0, p)


_ensure_concourse()

import concourse.bass as bass  # noqa: E402
import concourse.tile as tile  # noqa: E402
from concourse import mybir  # noqa: E402
from concourse.bass_utils import run_bass_kernel_spmd  # noqa: E402

# ---------------------------------------------------------------------------
# Walrus-compatibility shims.
#
# The walrus in this container rejects two things Tile's end-of-context tail
# emits: (a) the final Drain carrying >2 sem waits ("Too many sync wait
# commands"), and (b) EVENT_SEMAPHORE_RANGE_CLEAR ("ISA wrong length").
# Replace the tail with per-semaphore EventSemaphore instructions that wait
# for each sem's final value, then the normal all-engine barrier. A JSON-level
# pass additionally splits any instruction carrying more than MAX_WAITS sem
# waits into EventSemaphore wait carriers.
# ---------------------------------------------------------------------------

_MAX_WAITS = 1


def _patched_drain_and_barrier(self, tick_clock, wait_clock):
    nc = self.nc
    gc = tick_clock.global_clock
    sems = self.sems.allocated()  # proc idx -> SemaphoreHandle
    for proc in sorted(sems):
        h = sems[proc]
        if "DMA" not in h.name:
            continue  # engine sems are implied by stream completion
        final = int(gc[proc]) * 16
        if final > 0:
            nc.gpsimd.wait_ge(h, final)
    nc.all_engine_barrier()
    popped = nc._tile_sem_poison_stack.pop()
    assert popped is self._sem_poison


tile.TileContext._drain_and_barrier = _patched_drain_and_barrier

_orig_to_json_bytes = bass.Bass.to_json_bytes


def _to_json_bytes_compat(self):
    import orjson

    raw = _orig_to_json_bytes(self)
    d = orjson.loads(raw)
    changed = False
    for fn in d.get("functions", []):
        blocks = fn.get("basic_blocks") or fn.get("blocks") or []
        for bb in blocks:
            insts = bb.get("instructions", [])
            new_insts = []
            for inst in insts:
                waits = (inst.get("sync_info") or {}).get("on_wait") or []
                if len(waits) > _MAX_WAITS:
                    keep = waits[-_MAX_WAITS:]
                    excess = waits[:-_MAX_WAITS]
                    for i, wt in enumerate(excess):
                        new_insts.append(
                            {
                                "name": f"{inst['name']}_wsplit{i}",
                                "opcode": "EventSemaphore",
                                "engine": inst["engine"],
                                "ins": [],
                                "outs": [],
                                "debug": inst.get("debug"),
                                "sync_info": {"on_update": [], "on_wait": [wt]},
                            }
                        )
                    inst["sync_info"]["on_wait"] = keep
                    changed = True
                new_insts.append(inst)
            bb["instructions"] = new_insts
    return orjson.dumps(d) if changed else raw


bass.Bass.to_json_bytes = _to_json_bytes_compat

F32 = mybir.dt.float32
BF16 = mybir.dt.bfloat16
ALU = mybir.AluOpType
ACTF = mybir.ActivationFunctionType
AXL = mybir.AxisListType

B, SQ, SKV, D = 4, 2048, 8192, 1024
FACTOR = 4
N_CORES = 8
W_PER_CORE = B * SQ // N_CORES  # 1024 windows per core
P = 128  # windows per tile = SBUF partitions
LN_EPS = 1e-5
HALF = 512  # PSUM bank free-dim (f32)


def build_bass(n_tiles=W_PER_CORE // P, repeats=1, ablate=(), dma=None, skew=True,
               ssq_stage="mid", qk_first=False, batch_io=True, diag_eng="dve",
               dots="strick", wb="act", qstat_eng="act", psum_bufs=3,
               load_bufs=4, vstat_act=1, smalls_eng="pool", kv_batch=1):
    """repeats>1 wraps the body in a For_i hardware loop (timing NEFFs);
    repeats=1 is the straight-line correctness/production NEFF.
    ablate: timing-only probes that skip work (results become wrong):
      'ssq' | 'bn' | 'qstat' | 'outmm' | 'qkadd' | 'smalls'
    dma: queue map (q, k, v, out), entries 'sp' | 'act'.
    skew: software-pipelined stage emission (False = naive per-tile order).
    """
    if dma is None:
        dma = ("sp", "sp", "act", "act")
    dma_q, dma_k, dma_v, dma_o = dma
    nc = bass.Bass()
    if batch_io:
        assert n_tiles % 4 == 0
        q_d = nc.declare_dram_parameter(
            "q", [n_tiles // 4, P, 4, D], BF16, isOutput=False
        )
        o_d = nc.declare_dram_parameter(
            "out", [n_tiles // 4, P, 4, D], BF16, isOutput=True
        )
    else:
        q_d = nc.declare_dram_parameter("q", [n_tiles * P, D], BF16, isOutput=False)
        o_d = nc.declare_dram_parameter("out", [n_tiles * P, D], BF16, isOutput=True)
    if kv_batch == 2:
        k_d = nc.declare_dram_parameter(
            "k", [n_tiles // 2, P, 2, FACTOR, D], BF16, isOutput=False
        )
        v_d = nc.declare_dram_parameter(
            "v", [n_tiles // 2, P, 2, FACTOR, D], BF16, isOutput=False
        )
    else:
        k_d = nc.declare_dram_parameter(
            "k", [n_tiles * P, FACTOR, D], BF16, isOutput=False
        )
        v_d = nc.declare_dram_parameter(
            "v", [n_tiles * P, FACTOR, D], BF16, isOutput=False
        )
    id_d = nc.declare_dram_parameter("ident", [P, P], BF16, isOutput=False)

    lp = nc.allow_low_precision(reason="bf16 data/accums: rel_err gate is 2e-2")
    lp.__enter__()

    with tile.TileContext(nc) as tc:
        with (
            tc.tile_pool(name="qp", bufs=load_bufs) as qp,
            tc.tile_pool(name="kp", bufs=load_bufs) as kp,
            tc.tile_pool(name="vp", bufs=load_bufs) as vp,
            tc.tile_pool(name="qkp", bufs=3) as qkp,
            tc.tile_pool(name="outp", bufs=2) as outp,
            tc.tile_pool(name="scratch", bufs=2) as scratch,
            tc.tile_pool(name="smalls", bufs=4) as sm,
            tc.tile_pool(name="const", bufs=1) as cp,
            tc.tile_pool(name="psum", bufs=psum_bufs, space="PSUM") as pp,
            tc.tile_pool(name="qkpsum", bufs=2, space="PSUM") as qkpp,
        ):
            ident = cp.tile([P, P], BF16)
            nc.sync.dma_start(ident[:], id_d[:])

            from contextlib import nullcontext

            def eng(which):
                return {"sp": nc.sync, "act": nc.scalar}[which]

            st = {}  # t -> per-tile state dict

            def stage_load(t):
                rows = slice(t * P, (t + 1) * P)
                s = st.setdefault(t, {})
                if batch_io:
                    if t % 4 == 0:
                        qfat = qp.tile([P, 4, D], BF16, name="qfat", tag="q")
                        eng(dma_q).dma_start(qfat[:], q_d[t // 4])
                        st["qfat"] = qfat
                    s["q"] = st["qfat"][:, t % 4]
                else:
                    s["q"] = qp.tile([P, D], BF16, name="q", tag="q")
                    eng(dma_q).dma_start(s["q"][:], q_d[rows, :])
                if kv_batch == 2:
                    if t % 2 == 0:
                        kfat = kp.tile([P, 2, FACTOR, D], BF16, name="kfat", tag="k")
                        eng(dma_k).dma_start(kfat[:], k_d[t // 2])
                        st["kfat"] = kfat
                        vfat = vp.tile([P, 2, FACTOR, D], BF16, name="vfat", tag="v")
                        eng(dma_v).dma_start(vfat[:], v_d[t // 2])
                        st["vfat"] = vfat
                    s["k"] = st["kfat"][:, t % 2]
                    s["v"] = st["vfat"][:, t % 2]
                else:
                    s["k"] = kp.tile([P, FACTOR, D], BF16, name="k", tag="k")
                    eng(dma_k).dma_start(s["k"][:], k_d[rows, :, :])
                    s["v"] = vp.tile([P, FACTOR, D], BF16, name="v", tag="v")
                    eng(dma_v).dma_start(s["v"][:], v_d[rows, :, :])

            def stage_bn(t):
                """Front: DVE q+k adds then k/v bn_stats; ACT q-stats then
                S-squares (consuming the adds as they land)."""
                s = st[t]
                q_sb, k_sb, v_sb = s["q"], s["k"], s["v"]
                sum_q = sm.tile([P, 1], F32)
                ssq_q = sm.tile([P, 1], F32)
                if "qstat" in ablate:
                    nc.gpsimd.memset(sum_q[:], 0.0)
                    nc.gpsimd.memset(ssq_q[:], 1.0)
                elif qstat_eng == "dve":
                    qbn = sm.tile([P, 2, 6], F32, name="qbn")
                    for ch in range(2):
                        nc.vector.bn_stats(
                            qbn[:, ch], q_sb[:, ch * HALF : (ch + 1) * HALF]
                        )
                    qstats = sm.tile([P, 2], F32, name="qstats")
                    nc.vector.bn_aggr(
                        qstats[:], qbn[:].rearrange("p c x -> p (c x)")
                    )
                    # sum_q = mu*D ; ssq_q = (var + mu^2)*D
                    nc.gpsimd.tensor_scalar_mul(
                        sum_q[:], qstats[:, 0:1], float(D)
                    )
                    qmm = sm.tile([P, 1], F32, name="qmm")
                    nc.gpsimd.tensor_mul(qmm[:], qstats[:, 0:1], qstats[:, 0:1])
                    nc.gpsimd.tensor_scalar(
                        ssq_q[:], qstats[:, 1:2], qmm[:, 0:1], float(D),
                        ALU.add, ALU.mult,
                    )
                else:
                    dmpq = scratch.tile([P, D], BF16, tag="actdump")
                    nc.scalar.activation(
                        dmpq[:], q_sb[:], ACTF.Copy, accum_out=sum_q[:]
                    )
                    dmpq2 = scratch.tile([P, D], BF16, tag="actdump")
                    nc.scalar.activation(
                        dmpq2[:], q_sb[:], ACTF.Square, accum_out=ssq_q[:]
                    )
                s["sum_q"], s["ssq_q"] = sum_q, ssq_q

                def emit_qk_adds():
                    if dots in ("ttr", "pe"):
                        return
                    qk = qkp.tile([P, FACTOR, D], BF16, name="qk")
                    if "qkadd" not in ablate:
                        for f in range(FACTOR):
                            nc.vector.tensor_tensor(
                                qk[:, f], k_sb[:, f], q_sb[:], ALU.add
                            )
                    s["qk"] = qk

                def emit_ttr_dots():
                    rdot = sm.tile([P, FACTOR], F32, name="rdot_ttr")
                    if "ssq" not in ablate:
                        for f in range(FACTOR):
                            dmps = qkp.tile([P, D], BF16, tag="ttrdump")
                            nc.vector.tensor_tensor_reduce(
                                dmps[:], k_sb[:, f], q_sb[:], 1.0, 0.0,
                                ALU.mult, ALU.add,
                                accum_out=rdot[:, f : f + 1],
                            )
                    else:
                        nc.gpsimd.memset(rdot[:], 1.0)
                    s["rdot"] = rdot

                def emit_ssq():
                    if dots == "ttr":
                        emit_ttr_dots()
                        return
                    if dots == "pe":
                        S_h = sm.tile([P, 2, FACTOR], F32, name="S_h")
                        for f in range(FACTOR):
                            for h in range(2):
                                qk_ps = qkpp.tile([P, HALF], F32, tag="qkps")
                                nc.tensor.matmul(
                                    qk_ps[:],
                                    ident[:],
                                    q_sb[:, h * HALF : (h + 1) * HALF],
                                    start=True,
                                    stop=False,
                                )
                                nc.tensor.matmul(
                                    qk_ps[:],
                                    ident[:],
                                    k_sb[:, f, h * HALF : (h + 1) * HALF],
                                    start=False,
                                    stop=True,
                                )
                                dmh = scratch.tile([P, HALF], BF16, tag="acthalf")
                                nc.scalar.activation(
                                    dmh[:],
                                    qk_ps[:],
                                    ACTF.Square,
                                    accum_out=S_h[:, h, f : f + 1],
                                )
                        S = sm.tile([P, FACTOR], F32, name="S")
                        nc.vector.tensor_tensor(
                            S[:], S_h[:, 0], S_h[:, 1], ALU.add
                        )
                        s["S"] = S
                        return
                    S = sm.tile([P, FACTOR], F32, name="S")
                    if "ssq" not in ablate and "qkadd" not in ablate:
                        for f in range(FACTOR):
                            dmps = scratch.tile([P, D], BF16, tag="actdump")
                            nc.scalar.activation(
                                dmps[:], s["qk"][:, f], ACTF.Square,
                                accum_out=S[:, f : f + 1],
                            )
                    else:
                        nc.gpsimd.memset(S[:], 1.0)
                    s["S"] = S

                if qk_first:
                    emit_qk_adds()
                    if ssq_stage == "bn":
                        emit_ssq()
                s["emit_qk_adds"] = emit_qk_adds
                s["emit_ssq"] = emit_ssq

                bnst = sm.tile([P, 2, FACTOR, 2, 6], F32)
                stats = sm.tile([P, 2, FACTOR, 2], F32)  # (k/v, f, mean/var)
                if "bn" not in ablate:
                    vsums = sm.tile([P, 2, FACTOR], F32, name="vsums")
                    for i, x_sb in ((0, k_sb), (1, v_sb)):
                        for f in range(FACTOR):
                            if i == 1 and f >= FACTOR - vstat_act:
                                # v-stats via ACT 2-pass accumulate
                                dva = scratch.tile([P, D], BF16, tag="actdump")
                                nc.scalar.activation(
                                    dva[:], x_sb[:, f], ACTF.Copy,
                                    accum_out=vsums[:, 0, f : f + 1],
                                )
                                dvb = scratch.tile([P, D], BF16, tag="actdump")
                                nc.scalar.activation(
                                    dvb[:], x_sb[:, f], ACTF.Square,
                                    accum_out=vsums[:, 1, f : f + 1],
                                )
                                # mean = sum/D; var = ssq/D - mean^2
                                nc.gpsimd.tensor_scalar_mul(
                                    stats[:, 1, f, 0:1],
                                    vsums[:, 0, f : f + 1],
                                    1.0 / D,
                                )
                                vm2 = sm.tile([P, 1], F32, tag=f"vm2_{f}")
                                nc.gpsimd.tensor_mul(
                                    vm2[:],
                                    stats[:, 1, f, 0:1],
                                    stats[:, 1, f, 0:1],
                                )
                                nc.gpsimd.tensor_scalar(
                                    stats[:, 1, f, 1:2],
                                    vsums[:, 1, f : f + 1],
                                    1.0 / D,
                                    vm2[:, 0:1],
                                    ALU.mult,
                                    ALU.subtract,
                                )
                                continue
                            for ch in range(2):
                                nc.vector.bn_stats(
                                    bnst[:, i, f, ch],
                                    x_sb[:, f, ch * HALF : (ch + 1) * HALF],
                                )
                            nc.vector.bn_aggr(
                                stats[:, i, f],
                                bnst[:, i, f].rearrange("p c x -> p (c x)"),
                            )
                else:
                    nc.gpsimd.memset(stats[:], 0.5)
                s["stats"] = stats
                if not qk_first:
                    s["emit_qk_adds"]()
                    if ssq_stage == "bn":
                        s["emit_ssq"]()

            def stage_mid(t):
                """Small-tensor chain + diag builds (S/stats made last iter)."""
                s = st[t]
                if ssq_stage == "mid":
                    s["emit_ssq"]()
                stats = s["stats"]
                sum_q, ssq_q = s["sum_q"], s["ssq_q"]
                S = s.get("S")
                mu_k = stats[:, 0, :, 0]
                mu_v = stats[:, 1, :, 0]

                se = nc.vector if smalls_eng == "dve" else nc.gpsimd
                mu_q = sm.tile([P, 1], F32)
                se.tensor_scalar_mul(mu_q[:], sum_q[:], 1.0 / D)
                mmq = sm.tile([P, 1], F32)
                se.tensor_mul(mmq[:], mu_q[:], mu_q[:])

                var_all = sm.tile([P, 9], F32)
                se.tensor_scalar(
                    var_all[:, 0:1], ssq_q[:], 1.0 / D, mmq[:, 0:1],
                    ALU.mult, ALU.subtract,
                )
                se.tensor_copy(var_all[:, 1:5], stats[:, 0, :, 1])
                se.tensor_copy(var_all[:, 5:9], stats[:, 1, :, 1])
                veps = sm.tile([P, 9], F32)
                se.tensor_scalar_add(veps[:], var_all[:], LN_EPS)
                rall = sm.tile([P, 9], F32)
                nc.vector.reciprocal(rall[:], veps[:])
                a_all = sm.tile([P, 9], F32)
                nc.scalar.sqrt(a_all[:], rall[:])
                aq = a_all[:, 0:1]
                ak = a_all[:, 1:5]
                av = a_all[:, 5:9]

                if "smalls" not in ablate:
                    if dots == "ttr":
                        rdot = s["rdot"]
                    else:
                        mmk = sm.tile([P, FACTOR], F32)
                        se.tensor_mul(mmk[:], mu_k, mu_k)
                        vpm = sm.tile([P, FACTOR], F32)
                        se.tensor_tensor(
                            vpm[:], stats[:, 0, :, 1], mmk[:], ALU.add
                        )
                        # rdot = 0.5*(S - ssq_q) - 0.5*D*vpm
                        t_a = sm.tile([P, FACTOR], F32)
                        se.tensor_scalar(
                            t_a[:], S[:], ssq_q[:, 0:1], 0.5, ALU.subtract, ALU.mult
                        )
                        rdot = sm.tile([P, FACTOR], F32)
                        nc.vector.scalar_tensor_tensor(
                            rdot[:], vpm[:], -0.5 * D, t_a[:], ALU.mult, ALU.add
                        )
                    # w_f = aq*ak_f*(rdot_f - D*muq*muk_f); c_f = w_f*av_f
                    t1 = sm.tile([P, FACTOR], F32)
                    se.tensor_scalar(
                        t1[:], mu_k, mu_q[:, 0:1], None, ALU.mult
                    )
                    t2 = sm.tile([P, FACTOR], F32)
                    nc.vector.scalar_tensor_tensor(
                        t2[:], t1[:], -float(D), rdot[:], ALU.mult, ALU.add
                    )
                    u = sm.tile([P, FACTOR], F32)
                    se.tensor_scalar(u[:], ak, aq, None, ALU.mult)
                    w = sm.tile([P, FACTOR], F32)
                    se.tensor_mul(w[:], t2[:], u[:])
                    c = sm.tile([P, FACTOR], F32)
                    se.tensor_mul(c[:], w[:], av)
                    e = sm.tile([P, FACTOR], F32)
                    se.tensor_mul(e[:], c[:], mu_v)
                    neg_d = sm.tile([P, 1], F32)
                    nc.vector.tensor_reduce(
                        neg_d[:], e[:], AXL.X, ALU.add, negate=True
                    )
                else:
                    c = sm.tile([P, FACTOR], F32)
                    nc.gpsimd.tensor_scalar_mul(c[:], S[:], 1.0)
                    neg_d = sm.tile([P, 1], F32)
                    nc.gpsimd.memset(neg_d[:], 0.0)
                s["neg_d"] = neg_d

                diags = []
                diag_e = nc.vector if diag_eng == "dve" else nc.gpsimd
                for f in range(FACTOR):
                    dg = sm.tile([P, P], BF16, tag=f"diag{f}")
                    diag_e.tensor_scalar_mul(dg[:], ident[:], c[:, f : f + 1])
                    diags.append(dg)
                s["diags"] = diags

            def stage_out(t):
                """PE accumulation + ACT writeback + out DMA."""
                s = st.pop(t)
                q_sb, v_sb, diags = s["q"], s["v"], s["diags"]
                psum_t = pp.tile([P, 2, HALF], F32)
                if "outmm" not in ablate:
                    for h in range(2):
                        if wb != "dve":
                            nc.tensor.matmul(
                                psum_t[:, h],
                                ident[:],
                                q_sb[:, h * HALF : (h + 1) * HALF],
                                start=True,
                                stop=False,
                            )
                    for f in range(FACTOR):
                        for h in range(2):
                            nc.tensor.matmul(
                                psum_t[:, h],
                                diags[f][:],
                                v_sb[:, f, h * HALF : (h + 1) * HALF],
                                start=(wb == "dve" and f == 0),
                                stop=(f == FACTOR - 1),
                            )
                else:
                    for h in range(2):
                        nc.tensor.matmul(
                            psum_t[:, h],
                            diags[0][:],
                            v_sb[:, 0, h * HALF : (h + 1) * HALF],
                            start=True,
                            stop=True,
                        )
                if batch_io:
                    if t % 4 == 0:
                        st["outfat"] = outp.tile([P, 4, D], BF16, name="outfat")
                    out_sb = st["outfat"][:, t % 4]
                else:
                    out_sb = outp.tile([P, D], BF16, name="out_sb")
                if wb == "dve":
                    # out = (neg_d + psum_attn) + q in one DVE STT
                    nc.vector.scalar_tensor_tensor(
                        out_sb[:],
                        psum_t[:].rearrange("p c x -> p (c x)"),
                        s["neg_d"][:, 0:1],
                        q_sb[:],
                        ALU.add,
                        ALU.add,
                    )
                else:
                    nc.scalar.activation(
                        out_sb[:],
                        psum_t[:].rearrange("p c x -> p (c x)"),
                        ACTF.Identity,
                        bias=s["neg_d"][:],
                    )
                if batch_io:
                    if t % 4 == 3:
                        eng(dma_o).dma_start(o_d[t // 4], st["outfat"][:])
                else:
                    rows = slice(t * P, (t + 1) * P)
                    eng(dma_o).dma_start(o_d[rows, :], out_sb[:])

            loop_ctx = tc.For_i(0, repeats, 1) if repeats > 1 else nullcontext()
            with loop_ctx:
                if skew:
                    for i in range(n_tiles + 3):
                        if i < n_tiles:
                            stage_load(i)
                        if 0 <= i - 1 < n_tiles:
                            stage_bn(i - 1)
                        if 0 <= i - 2 < n_tiles:
                            stage_mid(i - 2)
                        if 0 <= i - 3 < n_tiles:
                            stage_out(i - 3)
                else:
                    for t in range(n_tiles):
                        stage_load(t)
                        stage_bn(t)
                        stage_mid(t)
                        stage_out(t)
    return nc


def make_in_map(q_core, k_core, v_core, batch_io=True, kv_batch=1):
    """Host-side per-core input prep shared by run()/test/sim: cast to bf16.
    batch_io permutes q to [T/4, P, 4, D] so 4 tiles load as one fat DMA;
    kv_batch=2 similarly permutes k/v to [T/2, P, 2, F, D]."""
    import ml_dtypes

    bf = ml_dtypes.bfloat16
    q = np.asarray(q_core, dtype=np.float32)
    k = np.asarray(k_core, dtype=np.float32)
    v = np.asarray(v_core, dtype=np.float32)
    T = q.shape[0] // P
    if batch_io:
        q = q.reshape(T // 4, 4, P, D).transpose(0, 2, 1, 3)
    if kv_batch == 2:
        k = k.reshape(T // 2, 2, P, FACTOR, D).transpose(0, 2, 1, 3, 4)
        v = v.reshape(T // 2, 2, P, FACTOR, D).transpose(0, 2, 1, 3, 4)
    return {
        "q": np.ascontiguousarray(q).astype(bf),
        "k": np.ascontiguousarray(k).astype(bf),
        "v": np.ascontiguousarray(v).astype(bf),
        "ident": np.eye(P, dtype=np.float32).astype(bf),
    }


def unpermute_out(out_core, batch_io=True):
    """Invert the batch_io out permutation: [T/4, P, 4, D] -> [T*P, D]."""
    if not batch_io:
        return np.asarray(out_core, dtype=np.float32)
    o = np.asarray(out_core, dtype=np.float32)
    g = o.shape[0]
    return o.transpose(0, 2, 1, 3).reshape(g * 4 * P, D)


_NC_CACHE = None


def _get_nc():
    global _NC_CACHE
    if _NC_CACHE is None:
        _NC_CACHE = build_bass()
    return _NC_CACHE


def _numpy_reference(query, key, value, ln_w, ln_b):
    def ln(x):
        mu = x.mean(-1, keepdims=True)
        var = ((x - mu) ** 2).mean(-1, keepdims=True)
        return (x - mu) / np.sqrt(var + LN_EPS) * ln_w + ln_b

    qn, kn, vn = ln(query), ln(key), ln(value)
    b, s, d = key.shape
    k_win = kn.reshape(b, s // FACTOR, FACTOR, d)
    wts = np.einsum("bsd,bsfd->bsf", qn, k_win).reshape(b, s)
    attn = (wts[:, :, None] * vn).reshape(b, s // FACTOR, FACTOR, d).sum(axis=2)
    return (query + attn).astype(np.float32)


def run(inputs, trace=False):
    """Returns (full_output, BassKernelResults-or-None)."""
    query = np.asarray(inputs["query"], dtype=np.float32)
    key = np.asarray(inputs["key"], dtype=np.float32)
    value = np.asarray(inputs["value"], dtype=np.float32)
    ln_w = np.asarray(inputs["ln_weight"], dtype=np.float32)
    ln_b = np.asarray(inputs["ln_bias"], dtype=np.float32)

    if not (np.all(ln_w == 1.0) and np.all(ln_b == 0.0)):
        # General-path fallback (setup_inputs always produces ones/zeros).
        return _numpy_reference(query, key, value, ln_w, ln_b), None

    sq_h = SQ // 2  # 1024 query rows per core
    skv_h = SKV // 2  # 4096 kv rows per core
    in_maps = []
    for cidx in range(N_CORES):
        bi, h = divmod(cidx, 2)
        in_maps.append(
            make_in_map(
                query[bi, h * sq_h : (h + 1) * sq_h],
                key[bi, h * skv_h : (h + 1) * skv_h].reshape(W_PER_CORE, FACTOR, D),
                value[bi, h * skv_h : (h + 1) * skv_h].reshape(W_PER_CORE, FACTOR, D),
            )
        )

    res = run_bass_kernel_spmd(
        _get_nc(), in_maps, core_ids=list(range(N_CORES)), trace=trace
    )
    out = np.empty((B, SQ, D), dtype=np.float32)
    for cidx in range(N_CORES):
        bi, h = divmod(cidx, 2)
        out[bi, h * sq_h : (h + 1) * sq_h] = unpermute_out(
            res.results[cidx]["out"]
        )
    return out, res


def kernel(**inputs) -> np.ndarray:
    out, _ = run(inputs)
    return out
